# revision 24
# baseline (speedup 1.0000x reference)
"""Trainium2 Bass kernel for EnhancedTransformerBlock on ragged graphs.

Layout: transposed activations [channels (partitions), nodes (free)].
Sharding: 64 graphs -> 8 cores x 8 slots by size-sorted rank (uniform slot
widths across cores for SPMD).

v2 design notes (cost-model driven):
- All "linear" matmuls (qkv, v, out_proj, ffn) run as fp8e4 DoubleRow
  (2 contraction rows per partition, 0.5 cyc/row) -> 4x fewer PE rows.
- Attention ctx is computed TRANSPOSED: ctx^T[q, ch] with P as stationary
  weights, so each (head, ktpair) costs only 33 output rows instead of qc.
  A 33rd "ones" channel appended to V yields sumexp in the same matmul.
- Softmax exp: one Act instruction per (slot, qchunk, ktile) over all 8
  heads ([128, 8*qc]); key-padding mask via per-partition bias (km).
  exp output written as fp8 directly into per-slot static pt buffers.
- ctx^T normalized per-partition (query-major) on DVE with a broadcast
  multiply, transposed back channel-major via a masked-diagonal matmul
  (the mask zeroes padded query columns, replacing the qm/BIG trick).
- Engine balance: Act = exp+gelu only; DVE = stats/applies/normalize;
  GpSimd(Pool) = k/v/ctx copies + residuals; PE everything else.
"""

import math
import numpy as np
import ml_dtypes

N_CORES = 8
B = 64
H = 256
NH = 8
HD = H // NH
EPS = 1e-5

import concourse.bass as bass
import concourse.bacc as bacc
import concourse.mybir as mybir
import concourse.tile as tile
from concourse.bass_utils import run_bass_kernel_spmd
from contextlib import ExitStack

F32 = mybir.dt.float32
F32R = mybir.dt.float32r
BF16 = mybir.dt.bfloat16
FP8 = mybir.dt.float8e4
AF = mybir.ActivationFunctionType
OP = mybir.AluOpType
DR = mybir.MatmulPerfMode.DoubleRow

NEG = -30000.0  # additive key mask (pre-exp); exp(-30000*SC... ) -> 0
SC = 1.0 / math.sqrt(HD)

NP_FP8 = ml_dtypes.float8_e4m3
NP_BF16 = ml_dtypes.bfloat16


def _plan(batch):
    batch = np.asarray(batch).astype(np.int64)
    counts = np.bincount(batch, minlength=B).astype(np.int64)
    starts = np.concatenate([[0], np.cumsum(counts)[:-1]])
    order = np.argsort(-counts, kind="stable")  # rank -> graph id
    NS = B // N_CORES  # slots per core
    Ms, slot_graph = [], np.zeros((N_CORES, NS), np.int64)
    for s in range(NS):
        blk = order[N_CORES * s: N_CORES * s + N_CORES]
        m = int(max(16, math.ceil(max(1, counts[blk].max()) / 16) * 16))
        Ms.append(m)
        for c in range(N_CORES):
            slot_graph[c, s] = blk[c]
    offs = np.concatenate([[0], np.cumsum(Ms)]).astype(np.int64)
    Rtot = int(offs[-1])
    nkt = [math.ceil(m / 128) for m in Ms]
    R = int(math.ceil(Rtot / 128) * 128)
    for s in range(NS):
        R = max(R, int(offs[s]) + 128 * nkt[s])
    return counts, starts, slot_graph, Ms, offs, nkt, Rtot, R


def _build(nc, Ms, offs, nkt, Rtot, R):
    NS = len(Ms)
    NKT = sum(nkt)
    # query chunks (<=128) per slot
    QCH = [[(qo, min(128, Ms[s] - qo)) for qo in range(0, Ms[s], 128)]
           for s in range(NS)]
    NQCH = sum(len(q) for q in QCH)
    # global index bases
    PTO = np.concatenate([[0], np.cumsum([n * 2048 for n in nkt])]).astype(int)
    VO = np.concatenate([[0], np.cumsum([n * 264 for n in nkt])]).astype(int)
    KMB = np.concatenate([[0], np.cumsum(nkt)]).astype(int)
    qidx = {}
    qi = 0
    for s in range(NS):
        for j in range(len(QCH[s])):
            qidx[(s, j)] = qi
            qi += 1
    PTW = int(PTO[-1])
    VW = int(VO[-1])
    NC = 128 + NKT  # cst columns

    # ---- DRAM tensors ----
    d_xt = nc.dram_tensor("xt", [2, 128, R], F32, kind="ExternalInput").ap()
    d_wqk = nc.dram_tensor("wqk", [128, 1024], FP8, kind="ExternalInput").ap()
    d_wv = nc.dram_tensor("wv", [128, 512], FP8, kind="ExternalInput").ap()
    d_wo = nc.dram_tensor("wo", [128, 512], FP8, kind="ExternalInput").ap()
    d_w1 = nc.dram_tensor("w1", [128, 2048], FP8, kind="ExternalInput").ap()
    d_w2 = nc.dram_tensor("w2", [128, 2048], F32R, kind="ExternalInput").ap()
    d_cst = nc.dram_tensor("cst", [128, NC], F32, kind="ExternalInput").ap()
    d_qid = nc.dram_tensor("qid", [128, NQCH * 128], BF16, kind="ExternalInput").ap()
    d_vimg = nc.dram_tensor("vimg", [128, VW], FP8, kind="ExternalInput").ap()
    d_zb = nc.dram_tensor("zb", [128, R], BF16, kind="ExternalInput").ap()
    d_ot = nc.dram_tensor("ot", [2, 128, R], F32, kind="ExternalOutput").ap()

    mm = nc.tensor.matmul

    with tile.TileContext(nc) as tc, ExitStack() as ctx:
        pers = ctx.enter_context(tc.tile_pool(name="pers", bufs=1))
        stat = ctx.enter_context(tc.tile_pool(name="stat", bufs=8))
        ctp = ctx.enter_context(tc.tile_pool(name="ctp", bufs=3))
        hgp = ctx.enter_context(tc.tile_pool(name="hgp", bufs=2))
        psS = ctx.enter_context(tc.tile_pool(name="psS", bufs=2, space="PSUM"))
        psC = ctx.enter_context(tc.tile_pool(name="psC", bufs=2, space="PSUM"))
        psL = ctx.enter_context(tc.tile_pool(name="psL", bufs=2, space="PSUM"))

        # ---- persistent SBUF ----
        xt = [pers.tile([128, R], F32, name=f"xt{i}", tag=f"xt{i}") for i in range(2)]
        xn = pers.tile([128, 2 * R], FP8, name="xn", tag="xn")
        qkt = [pers.tile([128, R], BF16, name=f"qk{m}", tag=f"qk{m}") for m in range(4)]
        qz = {(j, g): pers.tile([128, R], BF16, name=f"qz{j}{g}", tag=f"qz{j}{g}")
              for j in (1, 2, 3) for g in range(2)}
        vr = pers.tile([128, VW], FP8, name="vr", tag="vr")
        pt = pers.tile([128, PTW], FP8, name="pt", tag="pt")
        cxd = pers.tile([128, 2 * R], FP8, name="cxd", tag="cxd")
        x2 = [pers.tile([128, R], BF16, name=f"x2{i}", tag=f"x2{i}") for i in range(2)]
        wqk = pers.tile([128, 1024], FP8, name="wqk", tag="wqk")
        wv = pers.tile([128, 512], FP8, name="wv", tag="wv")
        wo = pers.tile([128, 512], FP8, name="wo", tag="wo")
        w1 = pers.tile([128, 2048], FP8, name="w1", tag="w1")
        w2 = pers.tile([128, 2048], F32R, name="w2", tag="w2")
        cst = pers.tile([128, NC], F32, name="cst", tag="cst")
        qid = pers.tile([128, NQCH * 128], BF16, name="qid", tag="qid")
        scl = [pers.tile([128, 16], F32, name=f"scl{i}", tag=f"scl{i}") for i in range(2)]
        sft = [pers.tile([128, 16], F32, name=f"sft{i}", tag=f"sft{i}") for i in range(2)]
        mv = [pers.tile([128, 32], F32, name=f"mv{i}", tag=f"mv{i}") for i in range(2)]

        # cst column layout
        co = 0
        def csl(n):
            nonlocal co
            a = co
            co += n
            return a
        c_qkb = csl(4)
        c_ob = csl(2)
        c_fb1 = csl(8)
        c_fb2 = csl(2)
        c_ga1 = csl(16)
        c_gA = csl(16)
        c_gB = csl(16)
        c_nw = [csl(16), csl(16)]
        c_nb = [csl(16), csl(16)]
        c_km = csl(NKT)
        assert co == NC

        def cc(base, i):  # one cst column as [128,1] AP
            return cst[:, base + i: base + i + 1]

        # ---- input DMAs ----
        nc.sync.dma_start(out=cst, in_=d_cst)
        nc.sync.dma_start(out=wqk, in_=d_wqk)
        nc.sync.dma_start(out=wv, in_=d_wv)
        nc.sync.dma_start(out=wo, in_=d_wo)
        nc.sync.dma_start(out=w1, in_=d_w1)
        nc.sync.dma_start(out=w2, in_=d_w2)
        nc.sync.dma_start(out=qid, in_=d_qid)
        nc.sync.dma_start(out=vr, in_=d_vimg)
        # xt in 4 pieces per ct, boundaries at slot offsets
        bnd = [0, int(offs[2]), int(offs[4]), int(offs[6]), R]
        for p in range(4):
            for ct in range(2):
                nc.sync.dma_start(out=xt[ct][:, bnd[p]:bnd[p + 1]],
                                  in_=d_xt[ct][:, bnd[p]:bnd[p + 1]])

        for key_, t_ in qz.items():
            nc.sync.dma_start(out=t_, in_=d_zb)
        # zero the dead tails (cols beyond Rtot) of fp8 activation buffers
        if R > Rtot:
            for ct in range(2):
                nc.gpsimd.memset(xn[:, ct * R + Rtot: (ct + 1) * R], 0.0)
                nc.gpsimd.memset(cxd[:, ct * R + Rtot: (ct + 1) * R], 0.0)

        # ---------- GraphNorm helpers ----------
        def gstats(src, widx, half):
            # bn stats into mv[widx]; col = 2s+ct; mean at col, var at 16+col
            mvw = mv[widx].rearrange("p (k c) -> p k c", k=2)
            for s in half:
                o = int(offs[s])
                for ct in range(2):
                    st6 = stat.tile([128, 6], F32, name="st6", tag="st6")
                    nc.vector.bn_stats(out=st6, in_=src[ct][:, o:o + Ms[s]])
                    c = 2 * s + ct
                    nc.vector.bn_aggr(out=mvw[:, :, c:c + 1], in_=st6)

        def gcorr(widx, half):
            # vectorized over the 8 cols of this half -> scale/shift
            c0 = 2 * half[0]
            n = 2 * len(half)
            mean_r = mv[widx][:, c0:c0 + n]
            var_r = mv[widx][:, 16 + c0:16 + c0 + n]
            ga1 = cst[:, c_ga1 + c0:c_ga1 + c0 + n]
            gA = cst[:, c_gA + c0:c_gA + c0 + n]
            gB = cst[:, c_gB + c0:c_gB + c0 + n]
            nwx = cst[:, c_nw[widx] + c0:c_nw[widx] + c0 + n]
            nbx = cst[:, c_nb[widx] + c0:c_nb[widx] + c0 + n]
            m2 = stat.tile([128, 8], F32, name="m2", tag="m2")
            nc.vector.tensor_mul(m2[:, :n], mean_r, mean_r)
            v1 = stat.tile([128, 8], F32, name="v1", tag="v1")
            nc.vector.tensor_mul(v1[:, :n], var_r, gA)
            var = stat.tile([128, 8], F32, name="var", tag="var")
            nc.vector.tensor_mul(var[:, :n], m2[:, :n], gB)
            nc.vector.tensor_add(var[:, :n], var[:, :n], v1[:, :n])
            lnv = stat.tile([128, 8], F32, name="lnv", tag="lnv")
            nc.scalar.activation(out=lnv[:, :n], in_=var[:, :n], func=AF.Ln)
            std = stat.tile([128, 8], F32, name="std", tag="std")
            nc.scalar.activation(out=std[:, :n], in_=lnv[:, :n], func=AF.Exp,
                                 scale=0.5)
            nc.vector.tensor_scalar_add(std[:, :n], std[:, :n], EPS)
            rstd = stat.tile([128, 8], F32, name="rstd", tag="rstd")
            scr = stat.tile([128, 8], F32, name="scr", tag="scr")
            nc.vector.reciprocal_approx_accurate(out=rstd[:, :n], in_=std[:, :n],
                                                 scratch=scr[:, :n])
            sc = scl[widx][:, c0:c0 + n]
            sh = sft[widx][:, c0:c0 + n]
            nc.vector.tensor_mul(sc, rstd[:, :n], nwx)
            mean = stat.tile([128, 8], F32, name="mean", tag="mean")
            nc.vector.tensor_mul(mean[:, :n], mean_r, ga1)
            nc.vector.tensor_mul(sh, mean[:, :n], sc)
            nc.vector.scalar_tensor_tensor(out=sh, in0=sh, scalar=-1.0, in1=nbx,
                                           op0=OP.mult, op1=OP.add)

        def gapply(src, widx, half):
            for s in half:
                o = int(offs[s])
                for ct in range(2):
                    c = 2 * s + ct
                    nc.gpsimd.tensor_scalar(
                        out=xn[:, ct * R + o: ct * R + o + Ms[s]],
                        in0=src[ct][:, o:o + Ms[s]],
                        scalar1=scl[widx][:, c:c + 1],
                        scalar2=sft[widx][:, c:c + 1],
                        op0=OP.mult, op1=OP.add)

        xnv = xn.rearrange("p (k r) -> p k r", k=2)
        cxv = cxd.rearrange("p (k r) -> p k r", k=2)
        wqkv = wqk.rearrange("p (k b) -> p k b", k=2)
        wvv = wv.rearrange("p (k b) -> p k b", k=2)
        wov = wo.rearrange("p (k b) -> p k b", k=2)
        w1v = w1.rearrange("p (k b) -> p k b", k=2)

        # ---------- qkv projection chunks ----------
        def qkv_chunk(o, w):
            for half2 in range(2):  # mt pair
                ps = psL.tile([128, 512], F32, name="psq", tag="psL")
                for mi in range(2):
                    mt = 2 * half2 + mi
                    mm(ps[:, 256 * mi:256 * mi + w],
                       wqkv[:, :, 128 * mt:128 * mt + 128],
                       xnv[:, :, o:o + w], start=True, stop=True, perf_mode=DR)
                for mi in range(2):
                    mt = 2 * half2 + mi
                    nc.vector.tensor_scalar_add(
                        qkt[mt][:, o:o + w], ps[:, 256 * mi:256 * mi + w],
                        cc(c_qkb, mt))
                    if half2 == 0:
                        for j in (1, 2, 3):
                            nc.gpsimd.tensor_copy(
                                qz[(j, mt)][32 * j:32 * j + 32, o:o + w],
                                qkt[mt][32 * j:32 * j + 32, o:o + w])

        # ---------- v rows per (slot, ktile) ----------
        def v_slot(s):
            for kt in range(nkt[s]):
                mkt = min(128, Ms[s] - 128 * kt)
                ko = int(offs[s]) + 128 * kt
                ps = psL.tile([128, 512], F32, name="psv", tag="psL")
                mm(ps[:mkt, :256], xnv[:, :, ko:ko + mkt], wvv[:, :, :256],
                   start=True, stop=True, perf_mode=DR)
                vo = int(VO[s]) + 264 * kt
                dst = vr[:mkt, vo:vo + 264].rearrange("p (h b) -> p h b", h=8)
                nc.vector.tensor_copy(dst[:, :, 0:32], ps[:mkt, :256])

        # ---------- attention per slot ----------
        def attn_slot(s):
            so = int(offs[s])
            nk = nkt[s]
            po0 = int(PTO[s])
            jq = 0  # qc128 chunk counter within slot
            for qo2 in range(0, Ms[s], 256):
                qc2 = min(256, Ms[s] - qo2)
                qb2 = so + qo2
                # scores + exp per (ktile, head-group)
                for kt in range(nk):
                    ko = so + 128 * kt
                    for g4 in range(2):
                        st = psS.tile([128, 1024], F32, name="st", tag="psS")
                        for hj in range(4):
                            h = 4 * g4 + hj
                            if hj == 0:
                                mm(st[:, 256 * hj:256 * hj + qc2],
                                   qkt[2 + g4][0:32, ko:ko + 128],
                                   qkt[g4][0:32, qb2:qb2 + qc2],
                                   start=True, stop=True)
                            else:
                                mm(st[:, 256 * hj:256 * hj + qc2],
                                   qkt[2 + g4][:, ko:ko + 128],
                                   qz[(hj, g4)][:, qb2:qb2 + qc2],
                                   start=True, stop=True)
                        stv = st.rearrange("p (h b) -> p h b", h=4)
                        po = po0 + 2048 * kt + 1024 * g4
                        ptv = pt[:, po:po + 1024].rearrange("p (h b) -> p h b", h=4)
                        nc.scalar.activation(
                            out=ptv[:, :, :qc2], in_=stv[:, :, :qc2], func=AF.Exp,
                            bias=cc(c_km, int(KMB[s]) + kt), scale=SC)
                # ctx per 128-query subchunk
                for qh in range((qc2 + 127) // 128):
                    qc = min(128, qc2 - 128 * qh)
                    qb = qb2 + 128 * qh
                    qcol = 128 * qh
                    cp = psC.tile([128, 264], F32, name="cp", tag="psC")
                    first = True
                    for h in range(8):
                        for jp in range(nk // 2):
                            po = po0 + 4096 * jp
                            lhs = pt[:, po:po + 4096].rearrange(
                                "p (k b) -> p k b", k=2)[
                                :, :, 256 * h + qcol:256 * h + qcol + qc]
                            vo = int(VO[s]) + 528 * jp
                            rhs = vr[:, vo:vo + 528].rearrange(
                                "p (k b) -> p k b", k=2)[:, :, 33 * h:33 * h + 33]
                            mm(cp[:qc, 33 * h:33 * h + 33], lhs, rhs,
                               start=first,
                               stop=(nk % 2 == 0 and jp == nk // 2 - 1),
                               perf_mode=DR, skip_group_check=True)
                            first = False
                        if nk % 2 == 1:
                            po = po0 + 2048 * (nk - 1) + 256 * h + qcol
                            lhs = pt[:, po:po + qc]
                            vo = int(VO[s]) + 264 * (nk - 1)
                            rhs = vr[:, vo + 33 * h: vo + 33 * h + 33]
                            mm(cp[:qc, 33 * h:33 * h + 33], lhs, rhs,
                               start=first, stop=True, skip_group_check=True)
                            first = False
                    # normalize (query-major): rec = 1/sumexp; ctn = ctx * rec
                    cpv = cp.rearrange("p (h b) -> p h b", h=8)
                    rec = stat.tile([128, 8], F32, name="rec", tag="rec")
                    nc.vector.reciprocal_approx_fast(
                        out=rec[:qc, :], in_=cpv[:qc, :, 32:33])
                    ctn = ctp.tile([128, 256], BF16, name="ctn", tag="ctn")
                    ctnv = ctn.rearrange("p (h b) -> p h b", h=8)
                    recb = rec[:qc, :, None].to_broadcast([qc, 8, 32])
                    nc.vector.tensor_tensor(
                        ctnv[:qc, :, :], cpv[:qc, :, 0:32], recb, OP.mult)
                    # transpose back channel-major via masked diagonal
                    qi = (qidx[(s, jq)]) * 128
                    jq += 1
                    for ct in range(2):
                        tp = psS.tile([128, 1024], F32, name="tp", tag="psS")
                        mm(tp[:, :qc], ctn[:qc, 128 * ct:128 * ct + 128],
                           qid[:qc, qi:qi + qc], start=True, stop=True)
                        nc.vector.tensor_copy(
                            cxd[:, ct * R + qb: ct * R + qb + qc], tp[:128, :qc])

        # ---------- out_proj + residual ----------
        def oproj_piece(o, w):
            ps = psL.tile([128, 512], F32, name="pso", tag="psL")
            for ct in range(2):
                mm(ps[:, 256 * ct:256 * ct + w],
                   wov[:, :, 128 * ct:128 * ct + 128],
                   cxv[:, :, o:o + w], start=True, stop=True, perf_mode=DR)
            for ct in range(2):
                nc.vector.scalar_tensor_tensor(
                    out=x2[ct][:, o:o + w], in0=ps[:, 256 * ct:256 * ct + w],
                    scalar=cc(c_ob, ct), in1=xt[ct][:, o:o + w],
                    op0=OP.add, op1=OP.add)

        # ---------- ffn + residual + out DMA (output reuses xt) ----------
        def ffn_chunk(o, w):
            hg = hgp.tile([128, 4096], F32R, name="hg", tag="hg")
            for mt in range(8):
                ps = psL.tile([128, 512], F32, name="psf", tag="psL")
                for o2 in range(0, w, 256):
                    wc = min(256, w - o2)
                    mm(ps[:, o2:o2 + wc], w1v[:, :, 128 * mt:128 * mt + 128],
                       xnv[:, :, o + o2:o + o2 + wc],
                       start=True, stop=True, perf_mode=DR)
                nc.scalar.activation(out=hg[:, 512 * mt:512 * mt + w],
                                     in_=ps[:, :w], func=AF.Gelu,
                                     bias=cc(c_fb1, mt))
            for ct in range(2):
                for o2 in range(0, w, 256):
                    wc = min(256, w - o2)
                    ps2 = psL.tile([128, 512], F32, name="psf2", tag="psL")
                    for jt in range(8):
                        mm(ps2[:, :wc],
                           w2[:, 256 * jt + 128 * ct:256 * jt + 128 * ct + 128],
                           hg[:, 512 * jt + o2:512 * jt + o2 + wc],
                           start=(jt == 0), stop=(jt == 7))
                    nc.vector.scalar_tensor_tensor(
                        out=xt[ct][:, o + o2:o + o2 + wc], in0=ps2[:, :wc],
                        scalar=cc(c_fb2, ct), in1=x2[ct][:, o + o2:o + o2 + wc],
                        op0=OP.add, op1=OP.add)
                    nc.sync.dma_start(out=d_ot[ct][:, o + o2:o + o2 + wc],
                                      in_=xt[ct][:, o + o2:o + o2 + wc])

        # ================= emission =================
        H0 = [0, 1, 2, 3]
        H1 = [4, 5, 6, 7]
        mid = int(offs[4])

        gstats(xt, 0, H0)
        gcorr(0, H0)
        gapply(xt, 0, H0)
        gstats(xt, 0, H1)
        gcorr(0, H1)
        gapply(xt, 0, H1)

        # qkv: chunks of 256 over [0, R); H0 part first
        cuts0 = list(range(0, mid - 255, 256))
        for o in cuts0:
            qkv_chunk(o, 256)
        for s in H0:
            v_slot(s)
        rest0 = cuts0[-1] + 256 if cuts0 else 0
        cuts1 = list(range(rest0, R, 256))
        for o in cuts1:
            qkv_chunk(o, min(256, R - o))
        for s in H1:
            v_slot(s)

        def opg2(half):
            lo = 0 if half is H0 else mid
            hi = mid if half is H0 else R
            o = lo
            while o < hi:
                w = min(256, hi - o)
                oproj_piece(o, w)
                o += w
            gstats(x2, 1, half)
            gcorr(1, half)
            gapply(x2, 1, half)

        for s in H0:
            attn_slot(s)
        opg2(H0)
        for s in H1:
            attn_slot(s)
        opg2(H1)
        o = 0
        while o < R:
            w = min(512, R - o)
            ffn_chunk(o, w)
            o += w
    return nc


_CACHE = {}


def _prepare(inputs):
    x = np.asarray(inputs["x"], np.float32)
    batch = np.asarray(inputs["batch"]).astype(np.int64)
    counts, starts, slot_graph, Ms, offs, nkt, Rtot, R = _plan(batch)
    NS = len(Ms)
    NKT = sum(nkt)
    QCH = [[(qo, min(128, Ms[s] - qo)) for qo in range(0, Ms[s], 128)]
           for s in range(NS)]
    NQCH = sum(len(q) for q in QCH)
    VO = np.concatenate([[0], np.cumsum([n * 264 for n in nkt])]).astype(int)
    KMB = np.concatenate([[0], np.cumsum(nkt)]).astype(int)
    VW = int(VO[-1])
    NC = 128 + NKT

    in_proj_w = np.asarray(inputs["in_proj_w"], np.float32)
    in_proj_b = np.asarray(inputs["in_proj_b"], np.float32)
    out_proj_w = np.asarray(inputs["out_proj_w"], np.float32)
    out_proj_b = np.asarray(inputs["out_proj_b"], np.float32)
    ffn_w1 = np.asarray(inputs["ffn_w1"], np.float32)
    ffn_b1 = np.asarray(inputs["ffn_b1"], np.float32)
    ffn_w2 = np.asarray(inputs["ffn_w2"], np.float32)
    ffn_b2 = np.asarray(inputs["ffn_b2"], np.float32)

    # fold the v-branch input bias through out_proj (exact, linear)
    ob_eff = out_proj_b + out_proj_w @ in_proj_b[2 * H:3 * H]

    def drpack(wT, ktiles, m):
        # wT: [out m, in 128*ktiles] -> [128, ktiles*m] fp8 DoubleRow layout
        a = wT.T.reshape(ktiles, 128, m).transpose(1, 0, 2).reshape(128, ktiles * m)
        return np.ascontiguousarray(a.astype(NP_FP8))

    wqk = drpack(in_proj_w[:2 * H], 2, 512)
    wvp = drpack(in_proj_w[2 * H:], 2, 256)
    wop = drpack(out_proj_w, 2, 256)
    w1p = drpack(ffn_w1, 2, 1024)
    w2p = np.ascontiguousarray(
        ffn_w2.T.reshape(8, 128, 256).transpose(1, 0, 2).reshape(128, 2048)
        .astype(np.float32))

    # cst (per-core): see _build column layout
    qkb = in_proj_b[:2 * H].reshape(4, 128).T  # [128, 4]
    obc = ob_eff.reshape(2, 128).T
    fb1 = ffn_b1.reshape(8, 128).T
    fb2 = ffn_b2.reshape(2, 128).T
    nw1 = np.asarray(inputs["norm1_w"], np.float32).reshape(2, 128)
    nb1 = np.asarray(inputs["norm1_b"], np.float32).reshape(2, 128)
    nw2 = np.asarray(inputs["norm2_w"], np.float32).reshape(2, 128)
    nb2 = np.asarray(inputs["norm2_b"], np.float32).reshape(2, 128)

    def expand16(w2x):  # [2,128] -> [128,16] col=2s+ct
        out = np.zeros((128, 16), np.float32)
        for s in range(8):
            for ct in range(2):
                out[:, 2 * s + ct] = w2x[ct]
        return out

    xT = x.T
    xts = np.zeros((N_CORES, 2, 128, R), np.float32)
    cstv = np.zeros((N_CORES, 128, NC), np.float32)
    qidv = np.zeros((N_CORES, 128, NQCH * 128), NP_BF16)
    vimg = np.zeros((128, VW), NP_FP8)
    zb = np.zeros((128, R), NP_BF16)
    for s in range(NS):
        for kt in range(nkt[s]):
            for h in range(8):
                vimg[:, int(VO[s]) + 264 * kt + 33 * h + 32] = NP_FP8(1.0)

    for c in range(N_CORES):
        cstv[c, :, 0:4] = qkb
        cstv[c, :, 4:6] = obc
        cstv[c, :, 6:14] = fb1
        cstv[c, :, 14:16] = fb2
        cstv[c, :, 64:80] = expand16(nw1)
        cstv[c, :, 96:112] = expand16(nb1)
        cstv[c, :, 80:96] = expand16(nw2)
        cstv[c, :, 112:128] = expand16(nb2)
        qi = 0
        for s in range(NS):
            g = slot_graph[c, s]
            n = int(counts[g])
            st = int(starts[g])
            o = int(offs[s])
            if n > 0:
                blk = xT[:, st:st + n]
                xts[c, 0, :, o:o + n] = blk[:128]
                xts[c, 1, :, o:o + n] = blk[128:]
            ne = max(n, 1)
            inv_nm1 = 1.0 / max(ne - 1, 1)
            for ct in range(2):
                cstv[c, :, 16 + 2 * s + ct] = Ms[s] / ne                      # ga1
                cstv[c, :, 32 + 2 * s + ct] = Ms[s] * inv_nm1                 # gA
                cstv[c, :, 48 + 2 * s + ct] = Ms[s] * (1.0 - Ms[s] / ne) * inv_nm1  # gB
            for kt in range(nkt[s]):
                v = min(128, max(0, n - 128 * kt))
                col = 128 + int(KMB[s]) + kt
                cstv[c, :v, col] = 0.0
                cstv[c, v:, col] = NEG
            for (qo, qc) in QCH[s]:
                nval = max(0, min(qc, n - qo))
                for i in range(nval):
                    qidv[c, i, qi * 128 + i] = NP_BF16(1.0)
                qi += 1

    key = (tuple(Ms), R)
    if key not in _CACHE:
        nc = bacc.Bacc("TRN2", target_bir_lowering=False, debug=False,
                       num_devices=N_CORES)
        _build(nc, Ms, offs, nkt, Rtot, R)
        nc.compile()
        _CACHE[key] = nc
    nc = _CACHE[key]

    in_maps = []
    for c in range(N_CORES):
        in_maps.append({
            "xt": xts[c], "wqk": wqk, "wv": wvp, "wo": wop, "w1": w1p,
            "w2": w2p, "cst": np.ascontiguousarray(cstv[c]),
            "qid": np.ascontiguousarray(qidv[c]), "vimg": vimg,
            "zb": zb,
        })

    def unpack(outs):
        out = np.empty((x.shape[0], H), np.float32)
        for c in range(N_CORES):
            ot = outs[c]["ot"]  # [2, 128, R]
            full = np.concatenate([ot[0], ot[1]], axis=0)  # [256, R]
            for s in range(NS):
                g = slot_graph[c, s]
                n = int(counts[g])
                st = int(starts[g])
                o = int(offs[s])
                if n > 0:
                    out[st:st + n] = full[:, o:o + n].T
        return out

    return nc, in_maps, unpack


def kernel(**inputs):
    nc, in_maps, unpack = _prepare(inputs)
    res = run_bass_kernel_spmd(nc, in_maps, list(range(N_CORES)))
    return unpack(res.results)


def _traced_run(**inputs):
    """Cost-model timeline (single core) + warm wall-clock. Returns model ns."""
    import time
    nc, in_maps, unpack = _prepare(inputs)
    t0 = time.time()
    run_bass_kernel_spmd(nc, in_maps, list(range(N_CORES)))
    t1 = time.time()
    run_bass_kernel_spmd(nc, in_maps, list(range(N_CORES)))
    t2 = time.time()
    print(f"wall cold: {t1 - t0:.2f}s  warm: {t2 - t1:.2f}s")
    from concourse.timeline_sim import TimelineSim
    import trails.perfetto as _tp
    for _m in ("enable_explicit_ordering", "reserve_process_order",
               "reserve_thread_order", "set_process_order", "set_thread_order",
               "add_instant"):
        if not hasattr(_tp.LazyPerfetto, _m):
            setattr(_tp.LazyPerfetto, _m, lambda self, *a, **k: None)
    if not hasattr(_tp.LazyPerfetto, "add_counter"):
        def _add_counter(self, *a, **k):
            try:
                self.update_counter(*a, **k)
            except Exception:
                pass
        _tp.LazyPerfetto.add_counter = _add_counter
    tl = TimelineSim(nc, trace=True)
    total = tl.simulate()
    pf = tl.perfetto
    if callable(pf):
        pf = pf()
    if pf is not None:
        try:
            pf.save("/root/problem/tl.perfetto-trace")
        except Exception as e:
            print("perfetto dump failed:", e)
    return total
